# revision 6
# baseline (speedup 1.0000x reference)
"""Causal multi-head attention (B=4, H=16, S=2048, D=64) on 8 TRN2 NeuronCores.

Sharding: B*H = 64 heads, 8 heads per core (data/head parallel, no comms).

Per-core pipeline (per head):
  - DMA Q,K,V [2048,64] f32 -> SBUF, cast to bf16 (DVE)
  - transpose Q,K to d-major via PE pair-transposes ([128s, 2x64d] ->
    [128, 128] PSUM, the two s-tiles on partition halves), then flatten to
    [128, 2048] with the top 64 partitions zeroed (K=128 contraction pad):
    even s-tiles via DVE copy, odd s-tiles via partition-shift DMA
  - QK^T strips E^T[k, q] as N<=512 K=128 matmuls into PSUM (zero-padded
    rows contribute nothing; full-K matmuls pipeline with hidden LDWEIGHTS)
  - exp via ScalarE (scale=1/8 folded in), f32 PSUM -> bf16 SBUF strips
  - causal zeroing of each strip's diagonal tile via gpsimd affine_select
  - A@V with a ones-column appended to V: O[q,0:64] = sum_k A^T_k.T @ V_k,
    O[q,64] = softmax denominator; emitted with a 2-strip lag so the PE
    doesn't wait on ScalarE; normalize with VectorE reciprocal + scale
"""

import os
import sys

try:
    import concourse.bass as bass  # noqa: F401
except ImportError:
    sys.path.insert(0, "/opt/trn_rl_repo")

import numpy as np

import concourse.mybir as mybir
import concourse.tile as tile
from concourse import bacc
from concourse.bass_utils import run_bass_kernel_spmd
from concourse.masks import make_identity

B, H, S, D = 4, 16, 2048, 64
N_CORES = 8
HEADS = B * H
HPC = HEADS // N_CORES  # heads per core
P = 128
ST = S // P  # 16 s-tiles per head

F32 = mybir.dt.float32
BF16 = mybir.dt.bfloat16

SCALE = 1.0 / float(np.sqrt(D))

LAG = int(os.environ.get("K_LAG", "2"))      # A@V emission lag (strips)
XLAG = int(os.environ.get("K_XLAG", "5"))    # extra lag for late (big) A@V waves
ETBUFS = int(os.environ.get("K_ETBUFS", "2"))
TP_TAG = os.environ.get("K_TPTAG", "tp")
MASK_ENG = os.environ.get("K_MASK_ENG", "pool")   # trimask: dve | pool
CAST_ENG = os.environ.get("K_CAST_ENG", "pool")   # q/k/v casts: dve | pool
DVE_SHARE = float(os.environ.get("K_DVESHARE", "0.45"))  # exp cols on DVE
NSPLIT_ALL = int(os.environ.get("K_NSPLIT", "1"))

# Schraudolph fast-exp on DVE: exp(s*x) ~= bitcast_bf16(int16(x*FXA + FXB)).
# Max pointwise rel err ~4%; interleaved with exact ACT exp pieces the
# end-to-end output error stays ~7e-3 (measured vs oracle).
FXA = SCALE * 128.0 * float(np.log2(np.e))
FXB = 128.0 * 127.0 - 7.0


def build_nc(heads_per_core=HPC):
    nc = bacc.Bacc("TRN2", target_bir_lowering=False, debug=False,
                   num_devices=N_CORES)
    q_d = nc.dram_tensor("Q", [heads_per_core, S, D], F32, kind="ExternalInput")
    k_d = nc.dram_tensor("K", [heads_per_core, S, D], F32, kind="ExternalInput")
    v_d = nc.dram_tensor("V", [heads_per_core, S, D], F32, kind="ExternalInput")
    o_d = nc.dram_tensor("out", [heads_per_core, S, D], F32, kind="ExternalOutput")

    with tile.TileContext(nc) as tc:
        with (
            tc.tile_pool(name="const", bufs=1) as const,
            tc.tile_pool(name="stage", bufs=int(os.environ.get("K_PBUFS", "2"))) as stage,
            tc.tile_pool(name="bfp", bufs=int(os.environ.get("K_PBUFS", "2"))) as bfp,
            tc.tile_pool(name="tp", bufs=int(os.environ.get("K_PBUFS", "2"))) as tpool,
            tc.tile_pool(name="atp", bufs=int(os.environ.get("K_ATBUFS", "2"))) as atp,
            tc.tile_pool(name="osb", bufs=2) as osbp,
            tc.tile_pool(name="small", bufs=8) as small,
            tc.tile_pool(name="ps", bufs=1, space="PSUM") as ps,
        ):
            identity = const.tile([P, P], BF16, tag="ident")
            make_identity(nc, identity)
            trimask = None
            if MASK_ENG == "dve":
                # upper-triangular (incl. diagonal) ones: keep q >= k
                trimask = const.tile([P, P], BF16, tag="trimask")
                nc.gpsimd.memset(trimask, 1.0)
                nc.gpsimd.affine_select(
                    out=trimask, in_=trimask,
                    compare_op=mybir.AluOpType.is_ge,
                    fill=0.0, base=0,
                    pattern=[[1, P]], channel_multiplier=-1,
                )
            cast_eng = nc.gpsimd if CAST_ENG == "pool" else nc.vector
            # greedy engine split for exp pieces (columns to DVE fast-exp)
            exp_cols = {"dve": 0, "tot": 0}

            def emit_prep(h, nsplit=1):
                """Load + cast + transpose head h's operands. Returns the
                tiles the strip loop needs (qT, kT flat d-major; v_aug).
                nsplit>1 stages the s-range in pieces so early strips can
                start before the whole head is loaded (head-0 ramp)."""
                q_raw = stage.tile([P, ST, D], F32, tag="qraw")
                k_raw = stage.tile([P, ST, D], F32, tag="kraw")
                v_raw = stage.tile([P, ST, D], F32, tag="vraw")
                # d-major operands, K=64 contraction (no zero padding)
                qT3 = tpool.tile([D, ST, P], BF16, tag="qT3")
                kT3 = tpool.tile([D, ST, P], BF16, tag="kT3")
                q_bf = bfp.tile([P, ST, D], BF16, tag="qbf")
                k_bf = bfp.tile([P, ST, D], BF16, tag="kbf")
                npr = ST // 2  # transpose pair count
                splits = [(ST * i // nsplit, ST * (i + 1) // nsplit)
                          for i in range(nsplit)]
                # All loads issue first (no waits among them), then the
                # per-split chains PHASE-ORDERED across Q and K so the DVE
                # (in-order) runs both casts before the PSUM copies — K's
                # PE transposes overlap Q's DVE copies instead of waiting.
                for s0, s1 in splits:
                    for (raw, d_) in ((q_raw, q_d), (k_raw, k_d)):
                        nc.sync.dma_start(
                            out=raw[:, s0:s1, :],
                            in_=d_[h].rearrange("(b p) d -> p b d", p=P)[:, s0:s1, :])
                for si, (s0, s1) in enumerate(splits):
                    if si == 1 or nsplit == 1:
                        # defer V out of the first in-flight DMA window:
                        # concurrent loads complete together, and only Q/K
                        # gate the QK^T ramp (V is first needed by A@V)
                        nc.sync.dma_start(
                            out=v_raw, in_=v_d[h].rearrange("(b p) d -> p b d", p=P))
                    chain = (
                        (q_raw, q_bf, qT3, "qodd"),
                        (k_raw, k_bf, kT3, "kodd"),
                    )
                    p0, p1 = s0 // 2, s1 // 2
                    # phase 1: casts
                    for (raw, bf_, t3, ostag) in chain:
                        cast_eng.tensor_copy(bf_[:, s0:s1, :], raw[:, s0:s1, :])
                    # phase 2: PE pair-transposes; flatten to t3 [64, 16, 128]
                    # (d-major; K=64 contraction)
                    tps = []
                    for (raw, bf_, t3, ostag) in chain:
                        tp_ps = ps.tile([P, npr, P], BF16, tag=TP_TAG,
                                        bufs=ETBUFS if TP_TAG == "et" else 2,
                                        name="tp_ps")
                        tps.append(tp_ps)
                        for pr in range(p0, p1):
                            nc.tensor.transpose(
                                tp_ps[:, pr, :],
                                bf_[:, 2 * pr:2 * pr + 2, :].rearrange("p a d -> p (a d)"),
                                identity,
                            )
                    # phase 3: flatten copies + partition-shift DMAs
                    for tp_ps, (raw, bf_, t3, ostag) in zip(tps, chain):
                        # even s-tiles (partitions 0:64) straight to t3
                        nc.vector.tensor_copy(t3[:, 2 * p0:2 * p1:2, :],
                                              tp_ps[0:64, p0:p1, :])
                        # odd s-tiles (partitions 64:128): PSUM->SBUF, then
                        # partition-shift DMA into t3
                        odd = stage.tile([P, npr, P], BF16, tag=ostag, name="odd")
                        nc.vector.tensor_copy(odd[64:P, p0:p1, :],
                                              tp_ps[64:P, p0:p1, :])
                        nc.sync.dma_start(out=t3[:, 2 * p0 + 1:2 * p1:2, :],
                                          in_=odd[64:P, p0:p1, :])
                v_aug = bfp.tile([P, ST, D + 1], BF16, tag="vaug")
                cast_eng.tensor_copy(v_aug[:, :, 0:D], v_raw)
                cast_eng.memset(v_aug[:, :, D:D + 1], 1.0)
                return (qT3.rearrange("p t c -> p (t c)"),
                        kT3.rearrange("p t c -> p (t c)"), v_aug)

            # Per-head pipeline state, keyed by head; two heads live at once.
            state = {}

            def emit_strip(h, j):
                """QK^T strip j of head h, exp, causal mask."""
                st = state[h]
                qT, kT = st["qT"], st["kT"]
                W = S - P * j  # valid q columns for key-tile j
                at = atp.tile([P, W], BF16, tag=f"at{j}", name=f"at_{h}_{j}")
                st["strips"].append(at)

                # PSUM pieces of up to 1024 columns; matmul chunks <= 512
                off = 0
                pieces = []
                while off < W:
                    w = min(1024, W - off)
                    # short strips (W<=512, one bank) borrow the tp slots,
                    # which are idle outside prep — this keeps the et
                    # rotation free across head boundaries so the next
                    # head's QK^T can run during these strips' exps
                    if w <= 512 and TP_TAG != "et":
                        et = ps.tile([P, w], F32, tag=TP_TAG, bufs=2, name="et")
                    else:
                        et = ps.tile([P, w], F32, tag="et", bufs=ETBUFS, name="et")
                    for c0 in range(0, w, 512):
                        cw = min(512, w - c0)
                        qg = P * j + off + c0
                        nc.tensor.matmul(
                            et[:, c0:c0 + cw],
                            lhsT=kT[:, P * j:P * (j + 1)],
                            rhs=qT[:, qg:qg + cw],
                            start=True, stop=True,
                        )
                    pieces.append((et, off, w))
                    off += w

                for (et, off, w) in pieces:
                    # interleave exact exp (ACT) with fast-exp (DVE) per
                    # piece so the Scalar engine stops being the wall and
                    # every output row mixes exact/fast weights (error
                    # averaging; see module docstring)
                    use_dve = (exp_cols["dve"] + w * 0.5
                               < DVE_SHARE * (exp_cols["tot"] + w))
                    exp_cols["tot"] += w
                    if use_dve:
                        exp_cols["dve"] += w
                        nc.vector.tensor_scalar(
                            out=at[:, off:off + w].bitcast(mybir.dt.int16),
                            in0=et,
                            scalar1=FXA, scalar2=FXB,
                            op0=mybir.AluOpType.mult,
                            op1=mybir.AluOpType.add,
                        )
                    else:
                        nc.scalar.activation(
                            at[:, off:off + w], et,
                            mybir.ActivationFunctionType.Exp,
                            scale=SCALE,
                        )
                # causal mask inside the diagonal tile: zero where q < k
                if MASK_ENG == "dve":
                    nc.vector.tensor_mul(at[:, 0:P], at[:, 0:P], trimask)
                else:
                    nc.gpsimd.affine_select(
                        out=at[:, 0:P], in_=at[:, 0:P],
                        compare_op=mybir.AluOpType.is_ge,
                        fill=0.0, base=0,
                        pattern=[[1, P]], channel_multiplier=-1,
                    )

            def emit_av(h, jq):
                """A@V for q-tile jq of head h (strips 0..jq ready); groups
                of four q-tiles share one PSUM bank + one batched normalize;
                DMA the head's output out after its last group."""
                st = state[h]
                strips, v_aug, o_sb = st["strips"], st["v_aug"], st["o_sb"]
                if jq % 4 == 0:
                    st["o4"] = ps.tile([P, 4, D + 1], F32, tag="o",
                                       bufs=2, name="o4")
                o4 = st["o4"]
                for k in range(jq + 1):
                    nc.tensor.matmul(
                        o4[:, jq % 4, :],
                        lhsT=strips[k][:, P * (jq - k):P * (jq - k) + P],
                        rhs=v_aug[:, k, :],
                        start=(k == 0), stop=(k == jq),
                    )
                if jq % 4 == 3:
                    recip4 = small.tile([P, 4], F32, tag="recip")
                    nc.vector.reciprocal(
                        recip4,
                        o4[:, :, D:D + 1].rearrange("p a b -> p (a b)"),
                    )
                    rb = bass.AP(tensor=recip4.tensor, offset=recip4.offset,
                                 ap=[recip4.ap[0], recip4.ap[1], [0, D]])
                    nc.vector.tensor_tensor(
                        out=o_sb[:, jq - 3:jq + 1, :],
                        in0=o4[:, :, 0:D], in1=rb,
                        op=mybir.AluOpType.mult,
                    )
                    # stream this group of 4 q-tiles out right away
                    nc.sync.dma_start(
                        out=o_d[h].rearrange("(b p) d -> p b d", p=P)
                                  [:, jq - 3:jq + 1, :],
                        in_=o_sb[:, jq - 3:jq + 1, :],
                    )
                if jq == ST - 1:
                    del state[h]

            # One flattened software pipeline over (head, strip): the A@V
            # wave trails the QK^T/exp wave ACROSS head boundaries. Later
            # q-tiles get EXTRA lag: exp work per strip shrinks with j while
            # the A@V train grows with jq, so pushing the big trains into the
            # next head's long-exp slots keeps ScalarE from starving.
            tasks = [(h, j) for h in range(heads_per_core) for j in range(ST)]
            av_slot = {}
            for g_av, (h_av, j_av) in enumerate(tasks):
                av_slot[g_av] = g_av + LAG + (XLAG if j_av >= 12 else 0)
            qT0, kT0, v_aug0 = emit_prep(0, nsplit=int(os.environ.get("K_NSPLIT0", "2")))
            state[0] = {"qT": qT0, "kT": kT0, "v_aug": v_aug0, "strips": [],
                        "o_sb": osbp.tile([P, ST, D], F32, tag="osb", name="osb0")}
            av_next = 0
            for g, (h, j) in enumerate(tasks):
                emit_strip(h, j)
                if j == 8 and h + 1 < heads_per_core:
                    qTn, kTn, v_augn = emit_prep(h + 1, nsplit=NSPLIT_ALL)
                    state[h + 1] = {
                        "qT": qTn, "kT": kTn, "v_aug": v_augn, "strips": [],
                        "o_sb": osbp.tile([P, ST, D], F32, tag="osb",
                                          name=f"osb{h + 1}"),
                    }
                while av_next < len(tasks) and av_slot[av_next] <= g:
                    emit_av(*tasks[av_next])
                    av_next += 1
            while av_next < len(tasks):
                emit_av(*tasks[av_next])
                av_next += 1

    nc.compile()
    return nc


_NC_CACHE = {}


def _get_nc(heads_per_core=HPC):
    if heads_per_core not in _NC_CACHE:
        _NC_CACHE[heads_per_core] = build_nc(heads_per_core)
    return _NC_CACHE[heads_per_core]


def run_sharded(Q, K, V, heads_per_core=HPC, **run_kwargs):
    """Q, K, V: [HEADS-or-subset, S, D] f32 flattened over (B, H)."""
    nc = _get_nc(heads_per_core)
    n = heads_per_core
    in_maps = [
        {
            "Q": np.ascontiguousarray(Q[i * n:(i + 1) * n]),
            "K": np.ascontiguousarray(K[i * n:(i + 1) * n]),
            "V": np.ascontiguousarray(V[i * n:(i + 1) * n]),
        }
        for i in range(N_CORES)
    ]
    last_err = None
    for attempt in range(3):
        try:
            res = run_bass_kernel_spmd(nc, in_maps,
                                       core_ids=list(range(N_CORES)),
                                       **run_kwargs)
            out = np.concatenate(
                [np.asarray(res.results[i]["out"]) for i in range(N_CORES)],
                axis=0)
            return out, res
        except Exception as e:  # transient NRT_EXEC_UNIT_UNRECOVERABLE etc.
            last_err = e
            import time
            time.sleep(2.0)
    raise last_err


def kernel(Q, K, V, mask=None):
    Q = np.asarray(Q, dtype=np.float32).reshape(HEADS, S, D)
    K = np.asarray(K, dtype=np.float32).reshape(HEADS, S, D)
    V = np.asarray(V, dtype=np.float32).reshape(HEADS, S, D)
    out, _ = run_sharded(Q, K, V)
    return out.reshape(B, H, S, D)



# revision 9
# speedup vs baseline: 1.4896x; 1.4896x over previous
"""Causal multi-head attention (B=4, H=16, S=2048, D=64) on 8 TRN2 NeuronCores.

Sharding: B*H = 64 heads, 8 heads per core (data/head parallel, no comms).

Per-core pipeline (per head):
  - DMA Q,K,V [2048,64] f32 -> SBUF, cast to bf16 (DVE)
  - transpose Q,K to d-major via PE pair-transposes ([128s, 2x64d] ->
    [128, 128] PSUM, the two s-tiles on partition halves), then flatten to
    [128, 2048] with the top 64 partitions zeroed (K=128 contraction pad):
    even s-tiles via DVE copy, odd s-tiles via partition-shift DMA
  - QK^T strips E^T[k, q] as N<=512 K=128 matmuls into PSUM (zero-padded
    rows contribute nothing; full-K matmuls pipeline with hidden LDWEIGHTS)
  - exp via ScalarE (scale=1/8 folded in), f32 PSUM -> bf16 SBUF strips
  - causal zeroing of each strip's diagonal tile via gpsimd affine_select
  - A@V with a ones-column appended to V: O[q,0:64] = sum_k A^T_k.T @ V_k,
    O[q,64] = softmax denominator; emitted with a 2-strip lag so the PE
    doesn't wait on ScalarE; normalize with VectorE reciprocal + scale
"""

import os
import sys

try:
    import concourse.bass as bass  # noqa: F401
except ImportError:
    sys.path.insert(0, "/opt/trn_rl_repo")

import numpy as np

import concourse.mybir as mybir
import concourse.tile as tile
from concourse import bacc
from concourse.bass_utils import run_bass_kernel_spmd
from concourse.masks import make_identity

B, H, S, D = 4, 16, 2048, 64
N_CORES = 8
HEADS = B * H
HPC = HEADS // N_CORES  # heads per core
P = 128
ST = S // P  # 16 s-tiles per head

F32 = mybir.dt.float32
BF16 = mybir.dt.bfloat16

SCALE = 1.0 / float(np.sqrt(D))

LAG = int(os.environ.get("K_LAG", "2"))      # A@V emission lag (strips)
XLAG = int(os.environ.get("K_XLAG", "5"))    # extra lag for late (big) A@V waves
ETBUFS = int(os.environ.get("K_ETBUFS", "2"))
TP_TAG = os.environ.get("K_TPTAG", "tp")
MASK_ENG = os.environ.get("K_MASK_ENG", "pool")   # trimask: dve | pool
CAST_ENG = os.environ.get("K_CAST_ENG", "dve")    # q/k/v casts: dve | pool
DVE_SHARE = float(os.environ.get("K_DVESHARE", "0.35"))  # exp cols on DVE
KC = int(os.environ.get("K_CONTR", "128"))        # QK^T contraction: 128 | 64
NSPLIT_ALL = int(os.environ.get("K_NSPLIT", "1"))

# Schraudolph fast-exp on DVE: exp(s*x) ~= bitcast_bf16(int16(x*FXA + FXB)).
# Max pointwise rel err ~4%; interleaved with exact ACT exp pieces the
# end-to-end output error stays ~7e-3 (measured vs oracle).
FXA = SCALE * 128.0 * float(np.log2(np.e))
FXB = 128.0 * 127.0 - 7.0


def build_nc(heads_per_core=HPC):
    nc = bacc.Bacc("TRN2", target_bir_lowering=False, debug=False,
                   num_devices=N_CORES)
    q_d = nc.dram_tensor("Q", [heads_per_core, S, D], F32, kind="ExternalInput")
    k_d = nc.dram_tensor("K", [heads_per_core, S, D], F32, kind="ExternalInput")
    v_d = nc.dram_tensor("V", [heads_per_core, S, D], F32, kind="ExternalInput")
    o_d = nc.dram_tensor("out", [heads_per_core, S, D], F32, kind="ExternalOutput")

    with tile.TileContext(nc) as tc:
        with (
            tc.tile_pool(name="const", bufs=1) as const,
            tc.tile_pool(name="stage", bufs=int(os.environ.get("K_PBUFS", "2"))) as stage,
            tc.tile_pool(name="bfp", bufs=int(os.environ.get("K_PBUFS", "2"))) as bfp,
            tc.tile_pool(name="tp", bufs=int(os.environ.get("K_PBUFS", "2"))) as tpool,
            tc.tile_pool(name="atp", bufs=int(os.environ.get("K_ATBUFS", "2"))) as atp,
            tc.tile_pool(name="osb", bufs=2) as osbp,
            tc.tile_pool(name="small", bufs=8) as small,
            tc.tile_pool(name="ps", bufs=1, space="PSUM") as ps,
        ):
            identity = const.tile([P, P], BF16, tag="ident")
            make_identity(nc, identity)
            trimask = None
            if MASK_ENG == "dve":
                # upper-triangular (incl. diagonal) ones: keep q >= k
                trimask = const.tile([P, P], BF16, tag="trimask")
                nc.gpsimd.memset(trimask, 1.0)
                nc.gpsimd.affine_select(
                    out=trimask, in_=trimask,
                    compare_op=mybir.AluOpType.is_ge,
                    fill=0.0, base=0,
                    pattern=[[1, P]], channel_multiplier=-1,
                )
            cast_eng = nc.gpsimd if CAST_ENG == "pool" else nc.vector
            # greedy engine split for exp pieces (columns to DVE fast-exp)
            exp_cols = {"dve": 0, "tot": 0}

            def emit_prep(h, nsplit=1):
                """Load + cast + transpose head h's operands. Returns the
                tiles the strip loop needs (qT, kT flat d-major; v_aug).
                nsplit>1 stages the s-range in pieces so early strips can
                start before the whole head is loaded (head-0 ramp)."""
                q_raw = stage.tile([P, ST, D], F32, tag="qraw")
                k_raw = stage.tile([P, ST, D], F32, tag="kraw")
                v_raw = stage.tile([P, ST, D], F32, tag="vraw")
                # d-major operands; KC=128 zero-pads the contraction (the
                # (128,128) PE tile path measures faster than (64,128))
                qT3 = tpool.tile([KC, ST, P], BF16, tag="qT3")
                kT3 = tpool.tile([KC, ST, P], BF16, tag="kT3")
                q_bf = bfp.tile([P, ST, D], BF16, tag="qbf")
                k_bf = bfp.tile([P, ST, D], BF16, tag="kbf")
                if KC == P and h < 2:  # pool slots keep zero top halves
                    nc.gpsimd.memset(qT3[64:P, :, :], 0.0)
                    nc.gpsimd.memset(kT3[64:P, :, :], 0.0)
                npr = ST // 2  # transpose pair count
                splits = [(ST * i // nsplit, ST * (i + 1) // nsplit)
                          for i in range(nsplit)]
                # All loads issue first (no waits among them), then the
                # per-split chains PHASE-ORDERED across Q and K so the DVE
                # (in-order) runs both casts before the PSUM copies — K's
                # PE transposes overlap Q's DVE copies instead of waiting.
                for s0, s1 in splits:
                    for (raw, d_) in ((q_raw, q_d), (k_raw, k_d)):
                        nc.sync.dma_start(
                            out=raw[:, s0:s1, :],
                            in_=d_[h].rearrange("(b p) d -> p b d", p=P)[:, s0:s1, :])
                for si, (s0, s1) in enumerate(splits):
                    if si == 1 or nsplit == 1:
                        # defer V out of the first in-flight DMA window:
                        # concurrent loads complete together, and only Q/K
                        # gate the QK^T ramp (V is first needed by A@V)
                        nc.sync.dma_start(
                            out=v_raw, in_=v_d[h].rearrange("(b p) d -> p b d", p=P))
                    chain = (
                        (q_raw, q_bf, qT3, "qodd"),
                        (k_raw, k_bf, kT3, "kodd"),
                    )
                    p0, p1 = s0 // 2, s1 // 2
                    # phase 1: casts
                    for (raw, bf_, t3, ostag) in chain:
                        cast_eng.tensor_copy(bf_[:, s0:s1, :], raw[:, s0:s1, :])
                    # phase 2: PE pair-transposes; flatten to t3 [64, 16, 128]
                    # (d-major; K=64 contraction)
                    tps = []
                    for (raw, bf_, t3, ostag) in chain:
                        tp_ps = ps.tile([P, npr, P], BF16, tag=TP_TAG,
                                        bufs=ETBUFS if TP_TAG == "et" else 2,
                                        name="tp_ps")
                        tps.append(tp_ps)
                        for pr in range(p0, p1):
                            nc.tensor.transpose(
                                tp_ps[:, pr, :],
                                bf_[:, 2 * pr:2 * pr + 2, :].rearrange("p a d -> p (a d)"),
                                identity,
                            )
                    # phase 3: flatten copies + partition-shift DMAs
                    for tp_ps, (raw, bf_, t3, ostag) in zip(tps, chain):
                        # even s-tiles (partitions 0:64) straight to t3
                        nc.vector.tensor_copy(t3[0:64, 2 * p0:2 * p1:2, :],
                                              tp_ps[0:64, p0:p1, :])
                        # odd s-tiles (partitions 64:128): PSUM->SBUF, then
                        # partition-shift DMA into t3
                        odd = stage.tile([P, npr, P], BF16, tag=ostag, name="odd")
                        nc.vector.tensor_copy(odd[64:P, p0:p1, :],
                                              tp_ps[64:P, p0:p1, :])
                        nc.sync.dma_start(out=t3[0:64, 2 * p0 + 1:2 * p1:2, :],
                                          in_=odd[64:P, p0:p1, :])
                v_aug = bfp.tile([P, ST, D + 1], BF16, tag="vaug")
                cast_eng.tensor_copy(v_aug[:, :, 0:D], v_raw)
                cast_eng.memset(v_aug[:, :, D:D + 1], 1.0)
                return (qT3.rearrange("p t c -> p (t c)"),
                        kT3.rearrange("p t c -> p (t c)"), v_aug)

            # Per-head pipeline state, keyed by head; two heads live at once.
            state = {}

            def emit_strip(h, j):
                """QK^T strip j of head h, exp, causal mask."""
                st = state[h]
                qT, kT = st["qT"], st["kT"]
                W = S - P * j  # valid q columns for key-tile j
                at = atp.tile([P, W], BF16, tag=f"at{j}", name=f"at_{h}_{j}")
                st["strips"].append(at)

                # PSUM pieces of up to 1024 columns; matmul chunks <= 512
                off = 0
                pieces = []
                while off < W:
                    w = min(1024, W - off)
                    # short strips (W<=512, one bank) borrow the tp slots,
                    # which are idle outside prep — this keeps the et
                    # rotation free across head boundaries so the next
                    # head's QK^T can run during these strips' exps
                    if w <= 512 and TP_TAG != "et":
                        et = ps.tile([P, w], F32, tag=TP_TAG, bufs=2, name="et")
                    else:
                        et = ps.tile([P, w], F32, tag="et", bufs=ETBUFS, name="et")
                    for c0 in range(0, w, 512):
                        cw = min(512, w - c0)
                        qg = P * j + off + c0
                        nc.tensor.matmul(
                            et[:, c0:c0 + cw],
                            lhsT=kT[:, P * j:P * (j + 1)],
                            rhs=qT[:, qg:qg + cw],
                            start=True, stop=True,
                        )
                    pieces.append((et, off, w))
                    off += w

                for (et, off, w) in pieces:
                    # interleave exact exp (ACT) with fast-exp (DVE) per
                    # piece so the Scalar engine stops being the wall and
                    # every output row mixes exact/fast weights (error
                    # averaging; see module docstring)
                    use_dve = (exp_cols["dve"] + w * 0.5
                               < DVE_SHARE * (exp_cols["tot"] + w))
                    exp_cols["tot"] += w
                    if use_dve:
                        exp_cols["dve"] += w
                        nc.vector.tensor_scalar(
                            out=at[:, off:off + w].bitcast(mybir.dt.int16),
                            in0=et,
                            scalar1=FXA, scalar2=FXB,
                            op0=mybir.AluOpType.mult,
                            op1=mybir.AluOpType.add,
                        )
                    else:
                        nc.scalar.activation(
                            at[:, off:off + w], et,
                            mybir.ActivationFunctionType.Exp,
                            scale=SCALE,
                        )
                # causal mask inside the diagonal tile: zero where q < k
                if MASK_ENG == "dve":
                    nc.vector.tensor_mul(at[:, 0:P], at[:, 0:P], trimask)
                else:
                    nc.gpsimd.affine_select(
                        out=at[:, 0:P], in_=at[:, 0:P],
                        compare_op=mybir.AluOpType.is_ge,
                        fill=0.0, base=0,
                        pattern=[[1, P]], channel_multiplier=-1,
                    )

            def emit_av(h, jq):
                """A@V for q-tile jq of head h (strips 0..jq ready); groups
                of four q-tiles share one PSUM bank + one batched normalize;
                DMA the head's output out after its last group."""
                st = state[h]
                strips, v_aug, o_sb = st["strips"], st["v_aug"], st["o_sb"]
                if jq % 4 == 0:
                    st["o4"] = ps.tile([P, 4, D + 1], F32, tag="o",
                                       bufs=2, name="o4")
                o4 = st["o4"]
                for k in range(jq + 1):
                    nc.tensor.matmul(
                        o4[:, jq % 4, :],
                        lhsT=strips[k][:, P * (jq - k):P * (jq - k) + P],
                        rhs=v_aug[:, k, :],
                        start=(k == 0), stop=(k == jq),
                    )
                if jq % 4 == 3:
                    recip4 = small.tile([P, 4], F32, tag="recip")
                    nc.vector.reciprocal(
                        recip4,
                        o4[:, :, D:D + 1].rearrange("p a b -> p (a b)"),
                    )
                    rb = bass.AP(tensor=recip4.tensor, offset=recip4.offset,
                                 ap=[recip4.ap[0], recip4.ap[1], [0, D]])
                    nc.vector.tensor_tensor(
                        out=o_sb[:, jq - 3:jq + 1, :],
                        in0=o4[:, :, 0:D], in1=rb,
                        op=mybir.AluOpType.mult,
                    )
                    # stream this group of 4 q-tiles out right away
                    nc.sync.dma_start(
                        out=o_d[h].rearrange("(b p) d -> p b d", p=P)
                                  [:, jq - 3:jq + 1, :],
                        in_=o_sb[:, jq - 3:jq + 1, :],
                    )
                if jq == ST - 1:
                    del state[h]

            # One flattened software pipeline over (head, strip): the A@V
            # wave trails the QK^T/exp wave ACROSS head boundaries. Later
            # q-tiles get EXTRA lag: exp work per strip shrinks with j while
            # the A@V train grows with jq, so pushing the big trains into the
            # next head's long-exp slots keeps ScalarE from starving.
            tasks = [(h, j) for h in range(heads_per_core) for j in range(ST)]
            av_slot = {}
            for g_av, (h_av, j_av) in enumerate(tasks):
                av_slot[g_av] = g_av + LAG + (XLAG if j_av >= 12 else 0)
            qT0, kT0, v_aug0 = emit_prep(0, nsplit=int(os.environ.get("K_NSPLIT0", "2")))
            state[0] = {"qT": qT0, "kT": kT0, "v_aug": v_aug0, "strips": [],
                        "o_sb": osbp.tile([P, ST, D], F32, tag="osb", name="osb0")}
            av_next = 0
            for g, (h, j) in enumerate(tasks):
                emit_strip(h, j)
                if j == 8 and h + 1 < heads_per_core:
                    qTn, kTn, v_augn = emit_prep(h + 1, nsplit=NSPLIT_ALL)
                    state[h + 1] = {
                        "qT": qTn, "kT": kTn, "v_aug": v_augn, "strips": [],
                        "o_sb": osbp.tile([P, ST, D], F32, tag="osb",
                                          name=f"osb{h + 1}"),
                    }
                while av_next < len(tasks) and av_slot[av_next] <= g:
                    emit_av(*tasks[av_next])
                    av_next += 1
            while av_next < len(tasks):
                emit_av(*tasks[av_next])
                av_next += 1

    nc.compile()
    return nc


_NC_CACHE = {}


def _get_nc(heads_per_core=HPC):
    if heads_per_core not in _NC_CACHE:
        _NC_CACHE[heads_per_core] = build_nc(heads_per_core)
    return _NC_CACHE[heads_per_core]


def run_sharded(Q, K, V, heads_per_core=HPC, **run_kwargs):
    """Q, K, V: [HEADS-or-subset, S, D] f32 flattened over (B, H)."""
    nc = _get_nc(heads_per_core)
    n = heads_per_core
    in_maps = [
        {
            "Q": np.ascontiguousarray(Q[i * n:(i + 1) * n]),
            "K": np.ascontiguousarray(K[i * n:(i + 1) * n]),
            "V": np.ascontiguousarray(V[i * n:(i + 1) * n]),
        }
        for i in range(N_CORES)
    ]
    last_err = None
    for attempt in range(3):
        try:
            res = run_bass_kernel_spmd(nc, in_maps,
                                       core_ids=list(range(N_CORES)),
                                       **run_kwargs)
            out = np.concatenate(
                [np.asarray(res.results[i]["out"]) for i in range(N_CORES)],
                axis=0)
            return out, res
        except Exception as e:  # transient NRT_EXEC_UNIT_UNRECOVERABLE etc.
            last_err = e
            import time
            time.sleep(2.0)
    raise last_err


def kernel(Q, K, V, mask=None):
    Q = np.asarray(Q, dtype=np.float32).reshape(HEADS, S, D)
    K = np.asarray(K, dtype=np.float32).reshape(HEADS, S, D)
    V = np.asarray(V, dtype=np.float32).reshape(HEADS, S, D)
    out, _ = run_sharded(Q, K, V)
    return out.reshape(B, H, S, D)

